# revision 30
# baseline (speedup 1.0000x reference)
"""Multi-head attention (B=2, S=2048, D=1024, H=16) on 8 Trainium2 cores.

Sharding: core c handles heads {2c, 2c+1} of BOTH batches (tensor parallel
over heads).  Each core computes a partial output projection [2, S, D] in
bf16; the host sums the 8 partials per batch (the "all-reduce" after W_o
done host-side).  Splitting heads instead of batches lets every core use
the TRUE per-batch key count nkt_b = ceil(valid_len_b / 128) instead of
max over batches, and keeps the load perfectly balanced for any
valid_lens split.

All matmul operands are bf16; PSUM accumulation stays fp32.  The program
is specialized on (nkt0, nkt1[, vl_b==0]); key tiles >= nkt_b are fully
masked and contribute exactly zero to both the softmax numerator and
denominator (the V/ones columns are pre-multiplied by the key mask z), so
skipping them is exact.  Compiled variants are cached per signature.

The ScalarE exp stream is the critical resource (~1.15us per [128,1024]
ACTIVATE x 4*(nkt0+nkt1)); everything is arranged to keep it busy from
~8us to the end:
  - inputs stream column-block-major ([wqkv | X0 | X1] in dram), 8
    per-fc DMAs per 512-col wave so the wave spreads over 8 DMA engines;
    dispatches round-robin over the sync/gpsimd/vector queues so
    descriptor writes don't serialize the head of the pipe.
  - work is organized as 16 groups (batch, 512-wide q-quarter).  Per key
    tile: 2 row-tiled score matmuls (the 2 heads at base partitions
    0/64) -> one [128,1024] exp -> 2 PV matmuls of the previous tile
    accumulating into pp (PSUM: st double buffer 4 banks + pp 2 banks +
    2 spare = 8).  No separate stage-A/stage-B phases: PV chases the exp
    stream one tile behind, so the last group drains in ~5us.
  - all other PE work (V projection, batch-1 projections, later Q
    chunks) is emitted as fillers inside the kt loops, ordered by DMA
    arrival so a stalled filler never head-of-line-blocks the scores.
  - softmax divide: den row -> DMA-reshape [128,4] -> DVE recip -> DMA
    back -> gpsimd partition_broadcast -> attn.T = pp[0:64]*recip (bf16).
  - phase 4 (partial = attn @ W_o[:, rows].T) uses its own 2 PSUM banks
    and overlaps later groups; the final quarter's evictions run on the
    by-then-idle ScalarE instead of the DVE.
Edge case valid_len == 0: z = ones, nkt = 16, and qsb is zeroed after the
  Q projection -> E = 1 -> uniform attention over all keys, exactly
  matching the reference (softmax of an all -1e9 row).
"""

import sys

if "/opt/trn_rl_repo" not in sys.path:
    sys.path.insert(0, "/opt/trn_rl_repo")

import numpy as np
from contextlib import ExitStack

import concourse.bass as bass
import concourse.tile as tile
from concourse import bacc, mybir
from concourse import bass_utils

F32 = mybir.dt.float32
BF16 = mybir.dt.bfloat16
EXP = mybir.ActivationFunctionType.Exp

B, S, D = 2, 2048, 1024
H, DK = 16, 64
HPC = 2            # heads per core per batch
HC = HPC * DK      # head-group width = 128
WQKV = 3 * HC      # 384
N_CORES = 8
PT = 128           # partitions
NTT = S // PT      # 16 token tiles
NFC = D // PT      # 8 feature chunks
QW = 512           # q-quarter width
NQ = S // QW       # 4 quarters
_DBG = {}


def _emit(tc, xt, wo, zt, out, nkts, zvls):
    nc = tc.nc
    SKs = [nkts[b] * PT for b in range(B)]
    kws = [
        [min(512, SKs[b] - c * 512) for c in range((SKs[b] + 511) // 512)]
        for b in range(B)
    ]
    dmaq = [nc.sync, nc.gpsimd, nc.scalar]
    dmai = [0]

    def dma(dst, src, eng=None):
        # 3-way dispatch rotation: descriptor writes are ~0.6us each and
        # would otherwise serialize the head of the input stream
        (eng if eng is not None else dmaq[dmai[0] % 3]).dma_start(dst, src)
        dmai[0] += 1

    with ExitStack() as ctx:
        sb = ctx.enter_context(tc.tile_pool(name="sb", bufs=1))

        # ---- resident inputs, DMA'd in dependency order ----
        ztt = sb.tile([PT, 2 * NTT], F32, name="ztt")
        nc.sync.dma_start(ztt[:], zt[:])
        attnT = [sb.tile([PT, S], BF16, name=f"attnT{b}") for b in range(B)]
        # warm the exp activation-table (~2.7us load) before the scalar
        # queue picks up its share of DMA dispatches; the corner of attnT
        # it writes is overwritten by phase 2
        nc.scalar.activation(attnT[0][:, 0:2 * NTT], ztt[:], EXP)

        xws = [sb.tile([PT, WQKV + 2 * S], BF16, name=f"xw{fc}") for fc in range(NFC)]
        wts = [xws[fc][:, 0:WQKV] for fc in range(NFC)]
        xts = [
            [xws[fc][:, WQKV + b * S:WQKV + (b + 1) * S] for fc in range(NFC)]
            for b in range(B)
        ]
        for fc in range(NFC):  # W_q|W_k first: gates the very first matmuls
            dma(xws[fc][:, 0:2 * HC], xt[fc * PT:(fc + 1) * PT, 0:2 * HC])
        for fc in range(NFC):  # X0 col-block 0 as 16 half-partition DMAs so
            for h in range(2):  # it spreads over every DMA engine at once
                r0 = fc * PT + h * 64
                dma(
                    xws[fc][h * 64:h * 64 + 64, WQKV:WQKV + 512],
                    xt[r0:r0 + 64, WQKV:WQKV + 512],
                )
        for fc in range(NFC):  # W_v (needed a few us later by V fillers)
            dma(xws[fc][:, 2 * HC:WQKV], xt[fc * PT:(fc + 1) * PT, 2 * HC:WQKV])
        for cb in range(1, 4):  # rest of X0 in 512-col waves
            for fc in range(NFC):
                c0 = WQKV + cb * 512
                dma(xws[fc][:, c0:c0 + 512], xt[fc * PT:(fc + 1) * PT, c0:c0 + 512])
        for cb in range(2):    # X1 in 1024-col waves
            for fc in range(NFC):
                c0 = WQKV + S + cb * 1024
                dma(
                    xws[fc][:, c0:c0 + 1024],
                    xt[fc * PT:(fc + 1) * PT, c0:c0 + 1024],
                )
        wos = sb.tile([PT, D], BF16, name="wos")
        dma(wos[:], wo[:])

        # ---- resident intermediates ----
        qsb = [sb.tile([PT, S], BF16, name=f"qsb{b}") for b in range(B)]
        ksb = [sb.tile([PT, SKs[b]], BF16, name=f"ksb{b}") for b in range(B)]
        vzs = [
            [sb.tile([PT, nkts[b], DK + 1], BF16, name=f"vz{b}_{h}") for h in range(HPC)]
            for b in range(B)
        ]

        with tc.tile_pool(name="ps_s", bufs=1, space="PSUM") as pss, \
             tc.tile_pool(name="ps_p", bufs=1, space="PSUM") as ppp, \
             tc.tile_pool(name="etp", bufs=4) as etp, \
             tc.tile_pool(name="rpp", bufs=2) as rpp, \
             tc.tile_pool(name="bpp", bufs=2) as bpp, \
             tc.tile_pool(name="stg", bufs=3) as stg:

            # Producers are keyed lists of small granules (a few matmuls
            # each): the filler queue drip-feeds one granule per kt slot in
            # DMA-arrival order so a filler never stalls the exp stream for
            # long, and any consumer force-emits a prerequisite the queue
            # hasn't finished yet (tile tracks dependencies by emission
            # order, so a consumer must never precede its producer).
            emitters = {}
            progress = {}
            fillers = []
            outq = []
            state = {"pop": None, "npot": 0}

            def _advance(key):
                i = progress.get(key, 0)
                parts = emitters[key]
                if i >= len(parts):
                    return False
                progress[key] = i + 1
                parts[i]()
                return True

            def ensure(key):
                while _advance(key):
                    pass

            def one_pot(b, tt, ev, pool=None):
                # one token block of phase 4: partial = attn @ W_o.T slice.
                # Two [128,512] halves on a double-buffered 1-bank tag so the
                # matmul of one half overlaps the eviction of the other.
                so = stg.tile([PT, D], BF16, name="so", tag="so")
                for half in range(2):
                    pot = (pool or state["pop"]).tile(
                        [PT, 512], F32, name="pot", tag="po"
                    )
                    nc.tensor.matmul(
                        pot[:],
                        attnT[b][:, tt * PT:(tt + 1) * PT],
                        wos[:, half * 512:(half + 1) * 512],
                        start=True, stop=True,
                    )
                    ev(so[:, half * 512:(half + 1) * 512], pot[:])
                state["oq"] = 1 - state.get("oq", 0)
                (nc.sync if state["oq"] else nc.gpsimd).dma_start(
                    out[b * S + tt * PT:b * S + (tt + 1) * PT, :], so[:]
                )

            def filler_slot():
                while fillers:
                    key = fillers[0]
                    if _advance(key):
                        if progress[key] >= len(emitters[key]):
                            fillers.pop(0)
                        return
                    fillers.pop(0)
                if outq and state["pop"] is not None:
                    b, tt = outq.pop(0)
                    state["npot"] += 1
                    one_pot(b, tt, nc.vector.tensor_copy)

            def proj_parts(pq, b, w_off, dst, c0, w):
                # a 512-wide column block of Q.T or K.T as two granules
                hold = [None]

                def part(p):
                    def f():
                        if p == 0:
                            hold[0] = pq.tile([PT, 512], F32, name="pqk", tag="pqk")
                        for fc in range(p * 4, p * 4 + 4):
                            nc.tensor.matmul(
                                hold[0][:, 0:w],
                                wts[fc][:, w_off:w_off + HC],
                                xts[b][fc][:, c0:c0 + w],
                                start=(fc == 0), stop=(fc == NFC - 1),
                            )
                        if p == 1:
                            nc.vector.tensor_copy(dst[:, c0:c0 + w], hold[0][:, 0:w])
                    return f

                return [part(0), part(1)]

            def emit_1b_tt(pq, b, tt):
                # V token-major for one token tile, masked by z
                pvt = pq.tile([PT, 512], F32, name="pvt", tag="pqk")[:, 0:HC]
                for fc in range(NFC):
                    nc.tensor.matmul(
                        pvt[:],
                        xts[b][fc][:, tt * PT:(tt + 1) * PT],
                        wts[fc][:, 2 * HC:3 * HC],
                        start=(fc == 0), stop=(fc == NFC - 1),
                    )
                for h in range(HPC):
                    nc.vector.tensor_scalar_mul(
                        vzs[b][h][:, tt, 0:DK],
                        pvt[:, h * DK:(h + 1) * DK],
                        ztt[:, b * NTT + tt:b * NTT + tt + 1],
                    )

            def group(b, q):
                # one (batch, q-quarter): per key tile, scores -> exp -> PV
                # of the previous tile; then divide + attnT writeback.
                nkt = nkts[b]
                q0 = q * QW
                ensure(("zcol", b))
                ensure(("q", b, q))
                pps = [
                    ppp.tile([DK + 1, QW], F32, name=f"pp{j}", tag=f"pp{j}")
                    for j in range(2)
                ]
                ets = {}

                def pv(kt):
                    ensure(("v", b, kt))
                    for j in range(2):
                        nc.tensor.matmul(
                            pps[j][:],
                            vzs[b][j][:, kt, :],
                            ets[kt][:, j * QW:(j + 1) * QW],
                            start=(kt == 0), stop=(kt == nkt - 1),
                        )

                for kt in range(nkt):
                    if kt % 4 == 0 and kt // 4 < len(kws[b]):
                        ensure(("k", b, kt // 4))
                    stm = pss.tile([PT, 2 * QW], F32, name="st", tag=f"st{kt % 2}")
                    for j in range(2):
                        nc.tensor.matmul(
                            stm[:, j * QW:(j + 1) * QW],
                            ksb[b][j * DK:(j + 1) * DK, kt * PT:(kt + 1) * PT],
                            qsb[b][j * DK:(j + 1) * DK, q0:q0 + QW],
                            start=True, stop=True,
                        )
                    et = etp.tile([PT, 2 * QW], BF16, name="et", tag="et")
                    nc.scalar.activation(et[:], stm[:], EXP)
                    ets[kt] = et
                    if kt > 0:
                        pv(kt - 1)
                    filler_slot()
                pv(nkt - 1)
                for j in range(2):
                    po = j * DK
                    # 1/den straight off the PSUM den row (~51 ULP approx is
                    # plenty; den in [1e-3, 1e5] so no edge cases), broadcast
                    # across the 64 head partitions, and normalize pp into
                    # attnT without ever copying pp out first.
                    dc = rpp.tile([1, QW], F32, name="dc", tag=f"dc{j}")
                    nc.vector.tensor_copy(dc[:], pps[j][DK:DK + 1, :])
                    dr = rpp.tile([1, QW], F32, name="dr", tag=f"dr{j}")
                    nc.vector.reciprocal_approx_fast(dr[:], dc[:])
                    rb = bpp.tile([DK, QW], F32, name="rb", tag=f"rb{j}")
                    nc.gpsimd.partition_broadcast(rb[:], dr[:])
                    nc.vector.tensor_mul(
                        attnT[b][po:po + DK, q0:q0 + QW], pps[j][0:DK, :], rb[:]
                    )

            # ---- phase 1 window: pqk(2) + st(4) + pp(2) = 8 banks ----
            with tc.tile_pool(name="ps_qk", bufs=2, space="PSUM") as pq:
                def q_parts(b, c):
                    parts = proj_parts(pq, b, 0, qsb[b], c * 512, 512)
                    if c == 3 and zvls[b]:
                        parts.append(lambda: nc.vector.memset(qsb[b][:], 0.0))
                    return parts

                for b in range(B):
                    emitters[("zcol", b)] = [
                        (lambda b=b, h=h: nc.vector.tensor_copy(
                            vzs[b][h][:, :, DK],
                            ztt[:, b * NTT:b * NTT + nkts[b]],
                        ))
                        for h in range(HPC)
                    ]
                    for c in range(4):
                        emitters[("q", b, c)] = q_parts(b, c)
                    for c in range(len(kws[b])):
                        emitters[("k", b, c)] = proj_parts(
                            pq, b, HC, ksb[b], c * 512, kws[b][c]
                        )
                    for tt in range(nkts[b]):
                        emitters[("v", b, tt)] = [
                            lambda b=b, tt=tt: emit_1b_tt(pq, b, tt)
                        ]
                # zvls[b]: the memset granule must follow every Q chunk, so
                # emit all four chunks up front in that (rare) case
                for b in range(B):
                    if zvls[b]:
                        for c in range(4):
                            ensure(("q", b, c))

                # filler order ~ DMA arrival: V(b0) paced one per kt slot,
                # late Q(b0) chunks threaded between, then all of batch 1
                fillers += [("v", 0, tt) for tt in range(min(4, nkts[0]))]
                fillers += [("q", 0, 1)]
                fillers += [("v", 0, tt) for tt in range(4, min(8, nkts[0]))]
                fillers += [("q", 0, 2)]
                fillers += [("v", 0, tt) for tt in range(8, nkts[0])]
                fillers += [("q", 0, 3)]
                fillers += [("k", 1, c) for c in range(len(kws[1]))]
                fillers += [("q", 1, c) for c in range(4)]
                fillers += [("v", 1, tt) for tt in range(nkts[1])]
                fillers += [("zcol", 1)]

                group(0, 0)
                group(0, 1)
                group(0, 2)
                group(0, 3)
                for key in list(emitters):     # leftovers (small nkt0)
                    ensure(key)

            # ---- phase 2 window: po(2) + st(4) + pp(2) = 8 banks ----
            # pots are drip-fed one per kt slot (filler_slot) so they never
            # head-of-line-block the PE queue; a couple of ScalarE-evicted
            # pots at each group boundary use the exp stream's dance bubble
            sev = lambda so, pot: nc.scalar.copy(so, pot)
            with tc.tile_pool(name="ps_o", bufs=2, space="PSUM") as pop:
                state["pop"] = pop
                outq += [(0, tt) for tt in range(NTT)]
                for q in range(NQ):
                    group(1, q)
                    for _ in range(2):
                        if outq:
                            b, tt = outq.pop(0)
                            one_pot(b, tt, sev)
                    outq += [(1, tt) for tt in range(4 * q, 4 * (q + 1))]
                _DBG["slot_pots"] = state["npot"]
                _DBG["tail_pots"] = len(outq)
                for i, (b, tt) in enumerate(outq):
                    one_pot(b, tt, sev if i % 2 else nc.vector.tensor_copy)
                outq.clear()


def build(nkts=(NTT, NTT), zvls=(False, False)):
    nc = bacc.Bacc(
        "TRN2",
        target_bir_lowering=False,
        debug=False,
        enable_asserts=True,
        num_devices=N_CORES,
    )
    xt = nc.dram_tensor("xt", [D, WQKV + 2 * S], BF16, kind="ExternalInput").ap()
    wo = nc.dram_tensor("wo", [HC, D], BF16, kind="ExternalInput").ap()
    zt = nc.dram_tensor("zt", [PT, 2 * NTT], F32, kind="ExternalInput").ap()
    out = nc.dram_tensor("out", [2 * S, D], BF16, kind="ExternalOutput").ap()
    with tile.TileContext(nc) as tc:
        _emit(tc, xt, wo, zt, out, list(nkts), list(zvls))
    nc.compile()
    return nc


_NCS = {}


def _get_nc(sig):
    if sig not in _NCS:
        _NCS[sig] = build(sig[0], sig[1])
    return _NCS[sig]


def _sig_for(vls):
    nkts, zvls = [], []
    for v in vls:
        v = int(v)
        zvls.append(v <= 0)
        nkts.append(NTT if v <= 0 else min(NTT, (v + PT - 1) // PT))
    return (tuple(nkts), tuple(zvls))


def make_in_maps(X, valid_lens, W_q, W_k, W_v, W_o):
    import ml_dtypes

    bf16 = ml_dtypes.bfloat16
    X = np.asarray(X, dtype=np.float32)
    W_q = np.asarray(W_q, dtype=np.float32)
    W_k = np.asarray(W_k, dtype=np.float32)
    W_v = np.asarray(W_v, dtype=np.float32)
    W_o = np.asarray(W_o, dtype=np.float32)
    vls = np.asarray(valid_lens).astype(np.int64)
    zts = []
    for b in range(B):
        vl = int(vls[b])
        if vl > 0:
            z = (np.arange(S) < vl).astype(np.float32)
        else:
            z = np.ones(S, dtype=np.float32)
        zts.append(z.reshape(NTT, PT).T)
    ztfull = np.ascontiguousarray(np.concatenate(zts, axis=1))
    xtb = np.concatenate([X[0].T, X[1].T], axis=1)
    in_maps = []
    for c in range(N_CORES):
        rows = slice(c * HC, (c + 1) * HC)
        wqkv = np.concatenate(
            [W_q[rows].T * 0.125, W_k[rows].T, W_v[rows].T], axis=1
        )
        xtw = np.concatenate([wqkv, xtb], axis=1)
        in_maps.append({
            "xt": np.ascontiguousarray(xtw).astype(bf16),
            "wo": np.ascontiguousarray(W_o.T[rows]).astype(bf16),
            "zt": ztfull,
        })
    return in_maps


def combine(outs):
    acc = np.asarray(outs[0], np.float32)
    for o in outs[1:]:
        acc = acc + np.asarray(o, np.float32)
    return acc.reshape(B, S, D)


def kernel(X, valid_lens, W_q, W_k, W_v, W_o):
    vls = np.asarray(valid_lens).astype(np.int64)
    nc = _get_nc(_sig_for(vls))
    in_maps = make_in_maps(X, valid_lens, W_q, W_k, W_v, W_o)
    res = bass_utils.run_bass_kernel_spmd(nc, in_maps, core_ids=list(range(N_CORES)))
    return combine([r["out"] for r in res.results])


# revision 32
# speedup vs baseline: 1.1817x; 1.1817x over previous
"""Multi-head attention (B=2, S=2048, D=1024, H=16) on 8 Trainium2 cores.

Sharding: core c handles batch b = c//4 and head group g = c%4 (4 heads each).
Each core computes its heads' attention output and a partial output
projection [S, D] in bf16; the host sums the 4 partials per batch (the
"all-reduce" after W_o done host-side).

All matmul operands are bf16 (fp32 runs 2-pass LOW_HIGH on the PE); PSUM
accumulation stays fp32.  The program is specialized on NKT =
max_b ceil(valid_len_b / 128): key tiles >= NKT are fully masked and
contribute exactly zero to both the softmax numerator and denominator
(the V/ones columns are pre-multiplied by the key mask z), so skipping
them is exact.  Compiled variants are cached per NKT.

Per-core pipeline (PSUM is the scarce resource - 8 banks):
  phase 1a: Q.T and K.T per head pair; phase 1b: V token-major * z + z
  ones-column.  Phase 2 runs per (q-half, head-pair) group, software-
  pipelined: stage A (scores+exp) of group i overlaps stage B (PV+divide)
  of group i-1 on the PE.  Per kt: st_m = scores.T (row-tiled pair of
  64-contraction matmuls at base partitions 0/64), E.T = exp(st_m) parked
  in SBUF, then pp_j[0:64] += Vz_j.T @ E_j with the z ones-column giving
  the softmax denominator for free in row 64.  Divide: den row ->
  DMA-reshape [128,8] -> DVE recip -> DMA back -> gpsimd
  partition_broadcast -> attn.T = pp[0:64]*recip (bf16).  Phase 4
  (partial = attn @ W_o[:, rows].T) reuses the st PSUM tags so its first
  half overlaps the trailing stage-B groups.
Edge case valid_len == 0: host sets s_b = 0, z = ones, NKT = 16 -> E = 1
  -> uniform attention over all keys, exactly matching the reference.
"""

import sys

if "/opt/trn_rl_repo" not in sys.path:
    sys.path.insert(0, "/opt/trn_rl_repo")

import numpy as np
from contextlib import ExitStack

import concourse.bass as bass
import concourse.tile as tile
from concourse import bacc, mybir
from concourse import bass_utils

F32 = mybir.dt.float32
BF16 = mybir.dt.bfloat16
EXP = mybir.ActivationFunctionType.Exp

B, S, D = 2, 2048, 1024
H, DK = 16, 64
HPC = 4            # heads per core
HC = HPC * DK      # head-group width = 256
N_CORES = 8
PT = 128           # partitions
NTT = S // PT      # 16 token tiles
NFC = D // PT      # 8 feature chunks
NQC = S // 512     # 4 q-chunks of 512
QH = 1024          # phase-2 q-half width


def _emit(tc, xt, wo, zt, out, nkt):
    nc = tc.nc
    SK = nkt * PT                       # active key span
    kws = [min(512, SK - c * 512) for c in range((SK + 511) // 512)]
    with ExitStack() as ctx:
        sb = ctx.enter_context(tc.tile_pool(name="sb", bufs=1))

        # ---- resident inputs ----
        wts, xts = [], []
        for fc in range(NFC):
            xw = sb.tile([PT, S + 3 * HC], BF16, name=f"xw{fc}")
            nc.sync.dma_start(xw[:], xt[fc * PT:(fc + 1) * PT, :])
            xts.append(xw[:, 0:S])
            wts.append(xw[:, S:S + 3 * HC])
        wos = []
        for c in range(2):
            t = sb.tile([PT, D], BF16, name=f"wos{c}")
            nc.sync.dma_start(t[:], wo[c * PT:(c + 1) * PT, :])
            wos.append(t)
        ztt = sb.tile([PT, NTT], F32, name="ztt")
        nc.sync.dma_start(ztt[:], zt[:])

        # ---- resident intermediates ----
        qsb = [sb.tile([PT, S], BF16, name=f"qsb{p}") for p in range(2)]
        ksb = [sb.tile([PT, SK], BF16, name=f"ksb{p}") for p in range(2)]
        vzs = [sb.tile([PT, nkt, DK + 1], BF16, name=f"vz{h}") for h in range(HPC)]
        attnT = [sb.tile([PT, S], BF16, name=f"attnT{c}") for c in range(2)]
        nc.scalar.activation(attnT[0][:, 0:NTT], ztt[:], EXP)

        with tc.tile_pool(name="ps_s", bufs=1, space="PSUM") as pss, \
             tc.tile_pool(name="etp", bufs=min(nkt + 4, 16)) as etp, \
             tc.tile_pool(name="upp", bufs=2) as upp, \
             tc.tile_pool(name="rpp", bufs=2) as rpp, \
             tc.tile_pool(name="bpp", bufs=2) as bpp, \
             tc.tile_pool(name="stg", bufs=3) as stg:

            def emit_1a(pq, p):
                for off, dst, widths in (
                    (0, qsb[p], [512] * NQC),
                    (HC, ksb[p], kws),
                ):
                    pts = [
                        pq.tile([PT, 512], F32, name="pqk", tag="pqk")
                        for _ in widths
                    ]
                    for fc in range(NFC):
                        ws = wts[fc][:, off + p * PT:off + (p + 1) * PT]
                        c0 = 0
                        for c, w in enumerate(widths):
                            nc.tensor.matmul(
                                pts[c][:, 0:w],
                                ws,
                                xts[fc][:, c0:c0 + w],
                                start=(fc == 0), stop=(fc == NFC - 1),
                            )
                            c0 += w
                    c0 = 0
                    for c, w in enumerate(widths):
                        nc.vector.tensor_copy(dst[:, c0:c0 + w], pts[c][:, 0:w])
                        c0 += w

            def emit_1b_tt(pv, tt):
                pvt = pv.tile([PT, 512], F32, name="pvt", tag="pqk")[:, 0:HC]
                for fc in range(NFC):
                    nc.tensor.matmul(
                        pvt[:],
                        xts[fc][:, tt * PT:(tt + 1) * PT],
                        wts[fc][:, 2 * HC:3 * HC],
                        start=(fc == 0), stop=(fc == NFC - 1),
                    )
                for h in range(HPC):
                    nc.vector.tensor_scalar_mul(
                        vzs[h][:, tt, 0:DK],
                        pvt[:, h * DK:(h + 1) * DK],
                        ztt[:, tt:tt + 1],
                    )

            def emit_a_kt(g, kt):
                qh, p = g
                q0 = qh * QH
                pair = []
                for m in range(2):
                    stm = pss.tile([PT, QH], F32, name=f"st{m}", tag=f"st{m}")
                    for j in range(2):
                        nc.tensor.matmul(
                            stm[:, j * 512:(j + 1) * 512],
                            ksb[p][j * DK:(j + 1) * DK, kt * PT:(kt + 1) * PT],
                            qsb[p][j * DK:(j + 1) * DK, q0 + m * 512:q0 + (m + 1) * 512],
                            start=True, stop=True,
                        )
                    etm = etp.tile([PT, QH], BF16, name=f"et{m}", tag=f"et{m}")
                    nc.scalar.activation(etm[:], stm[:], EXP)
                    pair.append(etm)
                return pair

            def emit_a(g):
                return [emit_a_kt(g, kt) for kt in range(nkt)]

            def emit_b(psp, g, ets):
                qh, p = g
                q0 = qh * QH
                pps = [
                    psp.tile([DK + 1, QH], F32, name=f"pp{j}", tag=f"pp{j}")
                    for j in range(2)
                ]
                for kt in range(nkt):
                    for m in range(2):
                        for j in range(2):
                            nc.tensor.matmul(
                                pps[j][:, m * 512:(m + 1) * 512],
                                vzs[2 * p + j][:, kt, :],
                                ets[kt][m][:, j * 512:(j + 1) * 512],
                                start=(kt == 0), stop=(kt == nkt - 1),
                            )
                for j in range(2):
                    po = j * DK
                    u = upp.tile([DK + 1, QH], F32, name=f"u{j}", tag=f"u{j}")
                    nc.vector.tensor_copy(u[:], pps[j][:])
                    dv = rpp.tile([PT, QH // PT], F32, name="dv", tag=f"dv{j}")
                    nc.sync.dma_start(dv[:], u[DK:DK + 1, :])
                    nc.vector.reciprocal(dv[:], dv[:])
                    rr = rpp.tile([1, QH], F32, name="rr", tag=f"rr{j}")
                    nc.sync.dma_start(rr[:], dv[:])
                    rb = bpp.tile([DK, QH], F32, name="rb", tag=f"rb{j}")
                    nc.gpsimd.partition_broadcast(rb[:], rr[:])
                    nc.vector.tensor_mul(
                        attnT[p][po:po + DK, q0:q0 + QH], u[0:DK, :], rb[:]
                    )

            def emit_out(tts):
                for tt in tts:
                    pot = pss.tile([PT, D], F32, name="pot", tag=f"st{tt % 2}")
                    for c in range(2):
                        for half in range(2):
                            nc.tensor.matmul(
                                pot[:, half * 512:(half + 1) * 512],
                                attnT[c][:, tt * PT:(tt + 1) * PT],
                                wos[c][:, half * 512:(half + 1) * 512],
                                start=(c == 0), stop=(c == 1),
                            )
                    so = stg.tile([PT, D], BF16, name="so", tag="so")
                    if tt % 2 == 0:
                        nc.vector.tensor_copy(so[:], pot[:])
                    else:
                        nc.scalar.copy(so[:], pot[:])
                    nc.sync.dma_start(out[tt * PT:(tt + 1) * PT, :], so[:])

            with tc.tile_pool(name="ps_qk", bufs=4, space="PSUM") as pq:
                emit_1a(pq, 0)
                e00 = emit_a((0, 0))
                emit_1a(pq, 1)
                for tt in range(nkt):
                    emit_1b_tt(pq, tt)
                for h in range(HPC):
                    nc.vector.tensor_copy(vzs[h][:, :, DK], ztt[:, 0:nkt])
            with tc.tile_pool(name="ps_p", bufs=1, space="PSUM") as psp:
                e01 = emit_a((0, 1))
                emit_b(psp, (0, 0), e00)
                e10 = emit_a((1, 0))
                emit_b(psp, (0, 1), e01)
                e11 = emit_a((1, 1))
                emit_out(range(0, NTT // 2))
                emit_b(psp, (1, 0), e10)
                emit_b(psp, (1, 1), e11)
                emit_out(range(NTT // 2, NTT))


def build(nkt=NTT):
    nc = bacc.Bacc(
        "TRN2",
        target_bir_lowering=False,
        debug=False,
        enable_asserts=True,
        num_devices=N_CORES,
    )
    xt = nc.dram_tensor("xt", [D, S + 3 * HC], BF16, kind="ExternalInput").ap()
    wo = nc.dram_tensor("wo", [HC, D], BF16, kind="ExternalInput").ap()
    zt = nc.dram_tensor("zt", [PT, NTT], F32, kind="ExternalInput").ap()
    out = nc.dram_tensor("out", [S, D], BF16, kind="ExternalOutput").ap()
    with tile.TileContext(nc) as tc:
        _emit(tc, xt, wo, zt, out, nkt)
    nc.compile()
    return nc


_NCS = {}


def _get_nc(nkt):
    if nkt not in _NCS:
        _NCS[nkt] = build(nkt)
    return _NCS[nkt]


def _nkt_for(vls):
    nkts = []
    for v in vls:
        v = int(v)
        nkts.append(NTT if v <= 0 else min(NTT, (v + PT - 1) // PT))
    return max(nkts)


def make_in_maps(X, valid_lens, W_q, W_k, W_v, W_o):
    import ml_dtypes

    bf16 = ml_dtypes.bfloat16
    X = np.asarray(X, dtype=np.float32)
    W_q = np.asarray(W_q, dtype=np.float32)
    W_k = np.asarray(W_k, dtype=np.float32)
    W_v = np.asarray(W_v, dtype=np.float32)
    W_o = np.asarray(W_o, dtype=np.float32)
    vls = np.asarray(valid_lens).astype(np.int64)
    in_maps = []
    for c in range(N_CORES):
        b, g = divmod(c, 4)
        rows = slice(g * HC, (g + 1) * HC)
        vl = int(vls[b])
        s = 0.125 if vl > 0 else 0.0
        if vl > 0:
            z = (np.arange(S) < vl).astype(np.float32)
        else:
            z = np.ones(S, dtype=np.float32)
        wqkv = np.concatenate(
            [W_q[rows].T * s, W_k[rows].T, W_v[rows].T], axis=1
        )
        xtw = np.concatenate([X[b].T, wqkv], axis=1)
        in_maps.append({
            "xt": np.ascontiguousarray(xtw).astype(bf16),
            "wo": np.ascontiguousarray(W_o.T[rows]).astype(bf16),
            "zt": np.ascontiguousarray(z.reshape(NTT, PT).T),
        })
    return in_maps


def combine(outs):
    out = np.empty((B, S, D), dtype=np.float32)
    for b in range(B):
        out[b] = (
            np.asarray(outs[4 * b], np.float32)
            + np.asarray(outs[4 * b + 1], np.float32)
            + np.asarray(outs[4 * b + 2], np.float32)
            + np.asarray(outs[4 * b + 3], np.float32)
        )
    return out


def kernel(X, valid_lens, W_q, W_k, W_v, W_o):
    vls = np.asarray(valid_lens).astype(np.int64)
    nc = _get_nc(_nkt_for(vls))
    in_maps = make_in_maps(X, valid_lens, W_q, W_k, W_v, W_o)
    res = bass_utils.run_bass_kernel_spmd(nc, in_maps, core_ids=list(range(N_CORES)))
    return combine([r["out"] for r in res.results])
